# revision 11
# baseline (speedup 1.0000x reference)
"""Trainium2 Bass kernel for nn_Conv2d_71476845740806.

Reference semantics (buggy naive Conv2d):
  xsum = pad(input, 1).sum(batch)                  # (1, C, 258, 258)
  conv = conv2d(xsum, weight, stride=2, VALID)     # (1, K, 128, 128)
  vals = conv[0, :, :64, :64]                      # (K, 64, 64)
  out  = zeros(B, K, 128, 128); out[:, :, ::2, ::2] = vals  (batch-replicated)

Only window starts (2i, 2j), i,j in [0,64) are used -> only padded rows/cols
0..128 of the summed image matter -> only input rows/cols 0..127.

Device strategy (8 cores, SPMD):
  - Shard the 64 output rows: core q computes rows 8q..8q+7 for ALL K=128
    filters. Needs padded rows 16q..16q+16 (17 rows) x 129 cols, all b, c.
  - Host preps one combined per-core tensor xc[128, 8, 1170]:
    partitions 0..63 = (c, even padded rows 0..8 x 130 cols),
    partitions 64..127 = (c, odd padded rows 0..7 x 130 cols, zero-padded).
    Contiguous per (partition, batch) -> near-peak DMA efficiency.
  - Batch-sum via DVE tensor_add (casting to the matmul dtype on write).
  - Conv as 6 matmuls per accumulation group into one PSUM bank [128, 512]:
      3 x contract-128 (kernel rows dh=0,1 paired across partition halves)
      3 x contract-64  (dh=2, even partitions only, shifted one row)
    rhs AP does the stride-2 column access directly: [.., (8 rows), (64 cols step 2)].
  - DMA out per-core vals (128, 8*64); host scatters into the zero output.
"""

import ml_dtypes
import numpy as np

import concourse.bacc as bacc
import concourse.bass as bass
import concourse.mybir as mybir
from concourse import tile
from concourse.bass_utils import run_bass_kernel_spmd

F32 = mybir.dt.float32
F32R = mybir.dt.float32r
BF16 = mybir.dt.bfloat16

B, C, H, W = 8, 64, 256, 256
K = 128
NCORES = 8
ROWS_PER_CORE = 8          # output rows per core (64 total)
ER = 9                     # even padded rows per core
OR = 8                     # odd padded rows per core
WCOLS = 130                # stored padded cols 0..129 (used: 0..128)
PITCH = ER * WCOLS         # 1170 per-batch free pitch
NOUT = ROWS_PER_CORE * 64  # 512

# Matmul input dtype: "fp32" (exact, 4 cyc/row), "f32r" (1 cyc/row),
# "bf16" (1 cyc/row, HAM-warmable).
MM_DTYPE = "f32r"
# Sum strategy: "tree" = full sum then 6 matmuls; "quad" = 2 groups of 4
# batches (12 matmuls); "pair" = 4 groups of 2 batches (24 matmuls).
STRATEGY = "pair"
# Input DMA chunks (must divide 8): 4 = batch pairs, 8 = single batches.
NCHUNK = 8
# Dummy bf16 matmuls (on already-landed chunk-0 data) to lift the PE HAM
# clock gate to 8/8 before the real matmuls start. 0 = off.
WARMUP = 0

TRACE = False
LAST_EXEC_NS = None

_cache = {}


def _mm_np_dtype():
    return ml_dtypes.bfloat16 if MM_DTYPE == "bf16" else np.float32


def _emit_conv_matmuls(nc, wpt3, w2t3, src3, psum3, start, stop):
    """6 matmuls of the 3x3 stride-2 conv of src into psum (accumulating).

    src3: [128, ER, WCOLS] AP; partitions 0..63 = (c, even rows),
          64..127 = (c, odd rows; row index i is padded row 2i+1).
    """
    for dw in range(3):
        # dh=0 (even rows, row i) paired with dh=1 (odd rows, row i)
        nc.tensor.matmul(
            psum3[:, :, :],
            wpt3[:, dw, :],
            src3[:, 0:ROWS_PER_CORE, dw : dw + 128 : 2],
            start=(start and dw == 0),
            stop=False,
        )
    for dw in range(3):
        # dh=2: even rows, row i+1
        nc.tensor.matmul(
            psum3[0:K, :, :],
            w2t3[0:64, dw, :],
            src3[0:64, 1 : 1 + ROWS_PER_CORE, dw : dw + 128 : 2],
            start=False,
            stop=(stop and dw == 2),
        )


def _build_program():
    key = (MM_DTYPE, STRATEGY, NCHUNK, WARMUP)
    if key in _cache:
        return _cache[key]

    mm_dt = {"fp32": F32, "f32r": F32R, "bf16": BF16}[MM_DTYPE]
    w_dram_dt = BF16 if mm_dt is BF16 else F32

    nc = bacc.Bacc(None)
    xc = nc.declare_dram_parameter("xc", [128, B, PITCH], F32, isOutput=False)
    # One combined weight tensor: cols 0:384 = pair weights (dh=0/1 on
    # partition halves), cols 384:768 (partitions 0:64) = dh=2 weights.
    wc = nc.declare_dram_parameter("wc", [128, 2 * 3 * K], w_dram_dt, isOutput=False)
    out = nc.declare_dram_parameter("out", [K, NOUT], F32, isOutput=True)

    bpc = B // NCHUNK  # batches per DMA chunk

    with tile.TileContext(nc) as tc:
        with (
            tc.tile_pool(name="sbuf", bufs=1) as pool,
            tc.tile_pool(name="psum", bufs=1, space="PSUM") as ppool,
        ):
            wt = pool.tile([128, 2 * 3 * K], mm_dt, tag="wt")
            # f32r needs a rounding producer -> SWDGE cast DMA; bf16/fp32 are
            # plain HWDGE copies.
            wdma = nc.gpsimd if mm_dt is F32R else nc.sync
            wdma.dma_start(out=wt[:, :], in_=wc[:, :])
            wpt3 = wt[:, 0 : 3 * K].rearrange("p (a b) -> p a b", a=3)
            w2t3 = wt[:, 3 * K : 6 * K].rearrange("p (a b) -> p a b", a=3)

            staging = pool.tile([128, B * PITCH], F32, tag="staging")
            st3 = staging[:, :].rearrange("p (b x) -> p b x", b=B)
            xcr = xc[:, :, :]
            # All input chunks on ONE HWDGE queue: strict FIFO makes chunk i
            # complete before chunk i+1 starts, so the adds pipeline. (Two
            # queues interleave at packet granularity and everything lands
            # at the same late time.)
            for ch in range(NCHUNK):
                b0 = ch * bpc
                nc.sync.dma_start(
                    out=st3[:, b0 : b0 + bpc, :], in_=xcr[:, b0 : b0 + bpc, :]
                )

            psum = ppool.tile([K, NOUT], F32, tag="psum")
            psum3 = psum[:, :].rearrange("p (r w) -> p r w", r=ROWS_PER_CORE)

            if WARMUP:
                # Reads chunk-0 staging data reinterpreted as bf16 -> starts
                # as soon as the first chunk lands, keeps the PE busy (bf16
                # counts for HAM) until the real matmuls begin.
                dpsum = ppool.tile([K, NOUT], F32, tag="dpsum")
                wu_lhs = staging[:, 0:64].bitcast(BF16)
                wu_rhs = staging[:, 64:320].bitcast(BF16)
                for i in range(WARMUP):
                    nc.tensor.matmul(
                        dpsum[:, :], wu_lhs, wu_rhs,
                        start=(i == 0), stop=(i == WARMUP - 1),
                    )

            spair = pool.tile([128, 4 * PITCH], mm_dt if STRATEGY == "pair" else F32,
                              tag="spair")
            sp3 = spair[:, :].rearrange("p (b x) -> p b x", b=4)

            if STRATEGY == "pair":
                for bp in range(4):
                    nc.vector.tensor_add(
                        sp3[:, bp, :], st3[:, 2 * bp, :], st3[:, 2 * bp + 1, :]
                    )
                    src3 = sp3[:, bp, :].rearrange("p (r w) -> p r w", r=ER)
                    _emit_conv_matmuls(
                        nc, wpt3, w2t3, src3, psum3, start=(bp == 0), stop=(bp == 3)
                    )
            elif STRATEGY == "quad":
                quad = pool.tile([128, 2 * PITCH], mm_dt, tag="quad")
                q3 = quad[:, :].rearrange("p (b x) -> p b x", b=2)
                for h in range(2):
                    for bp in (2 * h, 2 * h + 1):
                        nc.vector.tensor_add(
                            sp3[:, bp, :], st3[:, 2 * bp, :], st3[:, 2 * bp + 1, :]
                        )
                    nc.vector.tensor_add(
                        q3[:, h, :], sp3[:, 2 * h, :], sp3[:, 2 * h + 1, :]
                    )
                    src3 = q3[:, h, :].rearrange("p (r w) -> p r w", r=ER)
                    _emit_conv_matmuls(
                        nc, wpt3, w2t3, src3, psum3, start=(h == 0), stop=(h == 1)
                    )
            else:  # tree
                t01 = pool.tile([128, PITCH], F32, tag="t01")
                t23 = pool.tile([128, PITCH], F32, tag="t23")
                xsum = pool.tile([128, PITCH], mm_dt, tag="xsum")
                for bp in range(2):
                    nc.vector.tensor_add(
                        sp3[:, bp, :], st3[:, 2 * bp, :], st3[:, 2 * bp + 1, :]
                    )
                nc.vector.tensor_add(t01[:, :], sp3[:, 0, :], sp3[:, 1, :])
                for bp in range(2, 4):
                    nc.vector.tensor_add(
                        sp3[:, bp, :], st3[:, 2 * bp, :], st3[:, 2 * bp + 1, :]
                    )
                nc.vector.tensor_add(t23[:, :], sp3[:, 2, :], sp3[:, 3, :])
                nc.vector.tensor_add(xsum[:, :], t01[:, :], t23[:, :])
                src3 = xsum[:, :].rearrange("p (r w) -> p r w", r=ER)
                _emit_conv_matmuls(nc, wpt3, w2t3, src3, psum3, start=True, stop=True)

            # Split evac + out-DMA into halves so the second half's PSUM read
            # overlaps the first half's DMA issue/receipt.
            outs = pool.tile([K, NOUT], F32, tag="outs")
            H2 = NOUT // 2
            nc.vector.tensor_copy(outs[:, 0:H2], psum[:, 0:H2])
            nc.sync.dma_start(out=out[:, 0:H2], in_=outs[:, 0:H2])
            nc.vector.tensor_copy(outs[:, H2:NOUT], psum[:, H2:NOUT])
            nc.sync.dma_start(out=out[:, H2:NOUT], in_=outs[:, H2:NOUT])

    nc.compile()
    _cache[key] = nc
    return nc


def _prep_inputs(input, weight):
    inp = np.ascontiguousarray(input, dtype=np.float32)
    w = np.ascontiguousarray(weight, dtype=np.float32)

    # Padded top-left region: P[r, w] = padded coord (orig r-1, w-1)
    P = np.zeros((B, C, 130, WCOLS), np.float32)
    P[:, :, 1:129, 1:129] = inp[:, :, :128, :128]
    Pc = np.ascontiguousarray(P.transpose(1, 0, 2, 3))  # (C, B, 130, WCOLS)

    wdt = _mm_np_dtype()
    t = [w[:, :, dh, :].transpose(1, 2, 0).reshape(-1, 3 * K) for dh in range(3)]
    wc_host = np.zeros((128, 6 * K), np.float32)
    wc_host[0:64, 0 : 3 * K] = t[0]
    wc_host[64:128, 0 : 3 * K] = t[1]
    wc_host[0:64, 3 * K : 6 * K] = t[2]
    wc_host = np.ascontiguousarray(wc_host.astype(wdt))

    in_maps = []
    for q in range(NCORES):
        r0 = 16 * q
        xcq = np.zeros((128, B, PITCH), np.float32)
        xcq[0:64] = Pc[:, :, r0 : r0 + 17 : 2, :].reshape(64, B, PITCH)
        xcq[64:128, :, 0 : OR * WCOLS] = Pc[:, :, r0 + 1 : r0 + 16 : 2, :].reshape(
            64, B, OR * WCOLS
        )
        in_maps.append({"xc": xcq, "wc": wc_host})
    return in_maps


def kernel(input, weight):
    global LAST_EXEC_NS
    nc = _build_program()
    in_maps = _prep_inputs(input, weight)
    res = run_bass_kernel_spmd(nc, in_maps, list(range(NCORES)), trace=TRACE)
    LAST_EXEC_NS = res.exec_time_ns

    vals = np.concatenate(
        [res.results[q]["out"].reshape(K, ROWS_PER_CORE, 64) for q in range(NCORES)],
        axis=1,
    )  # (K, 64, 64)
    out = np.zeros((B, K, 128, 128), np.float32)
    out[:, :, ::2, ::2] = vals[None]
    return out


# revision 12
# speedup vs baseline: 1.0266x; 1.0266x over previous
"""Trainium2 Bass kernel for nn_Conv2d_71476845740806.

Reference semantics (buggy naive Conv2d):
  xsum = pad(input, 1).sum(batch)                  # (1, C, 258, 258)
  conv = conv2d(xsum, weight, stride=2, VALID)     # (1, K, 128, 128)
  vals = conv[0, :, :64, :64]                      # (K, 64, 64)
  out  = zeros(B, K, 128, 128); out[:, :, ::2, ::2] = vals  (batch-replicated)

Only window starts (2i, 2j), i,j in [0,64) are used -> only padded rows/cols
0..128 of the summed image matter -> only input rows/cols 0..127.

Device strategy (8 cores, SPMD):
  - Shard the 64 output rows: core q computes rows 8q..8q+7 for ALL K=128
    filters. Needs padded rows 16q..16q+16 (17 rows) x 129 cols, all b, c.
  - Host preps one combined per-core tensor xc[128, 8, 1170]:
    partitions 0..63 = (c, even padded rows 0..8 x 130 cols),
    partitions 64..127 = (c, odd padded rows 0..7 x 130 cols, zero-padded).
    Contiguous per (partition, batch) -> near-peak DMA efficiency.
  - Batch-sum via DVE tensor_add (casting to the matmul dtype on write).
  - Conv as 6 matmuls per accumulation group into one PSUM bank [128, 512]:
      3 x contract-128 (kernel rows dh=0,1 paired across partition halves)
      3 x contract-64  (dh=2, even partitions only, shifted one row)
    rhs AP does the stride-2 column access directly: [.., (8 rows), (64 cols step 2)].
  - DMA out per-core vals (128, 8*64); host scatters into the zero output.
"""

import ml_dtypes
import numpy as np

import concourse.bacc as bacc
import concourse.bass as bass
import concourse.mybir as mybir
from concourse import tile
from concourse.bass_utils import run_bass_kernel_spmd

F32 = mybir.dt.float32
F32R = mybir.dt.float32r
BF16 = mybir.dt.bfloat16

B, C, H, W = 8, 64, 256, 256
K = 128
NCORES = 8
ROWS_PER_CORE = 8          # output rows per core (64 total)
ER = 9                     # even padded rows per core
OR = 8                     # odd padded rows per core
WCOLS = 130                # stored padded cols 0..129 (used: 0..128)
PITCH = ER * WCOLS         # 1170 per-batch free pitch
NOUT = ROWS_PER_CORE * 64  # 512

# Matmul input dtype: "fp32" (exact, 4 cyc/row), "f32r" (1 cyc/row),
# "bf16" (1 cyc/row, HAM-warmable).
MM_DTYPE = "f32r"
# Sum strategy: "tree" = full sum then 6 matmuls; "quad" = 2 groups of 4
# batches (12 matmuls); "pair" = 4 groups of 2 batches (24 matmuls).
STRATEGY = "pair"
# Input DMA chunks (must divide 8): 4 = batch pairs, 8 = single batches.
NCHUNK = 8
# Dummy bf16 matmuls (on already-landed chunk-0 data) to lift the PE HAM
# clock gate to 8/8 before the real matmuls start. 0 = off.
WARMUP = 0

TRACE = False
LAST_EXEC_NS = None

_cache = {}


def _mm_np_dtype():
    return ml_dtypes.bfloat16 if MM_DTYPE == "bf16" else np.float32


def _emit_conv_matmuls(nc, wpt3, w2t3, src3, psum3, start, stop):
    """6 matmuls of the 3x3 stride-2 conv of src into psum (accumulating).

    src3: [128, ER, WCOLS] AP; partitions 0..63 = (c, even rows),
          64..127 = (c, odd rows; row index i is padded row 2i+1).
    """
    for dw in range(3):
        # dh=0 (even rows, row i) paired with dh=1 (odd rows, row i)
        nc.tensor.matmul(
            psum3[:, :, :],
            wpt3[:, dw, :],
            src3[:, 0:ROWS_PER_CORE, dw : dw + 128 : 2],
            start=(start and dw == 0),
            stop=False,
        )
    for dw in range(3):
        # dh=2: even rows, row i+1
        nc.tensor.matmul(
            psum3[0:K, :, :],
            w2t3[0:64, dw, :],
            src3[0:64, 1 : 1 + ROWS_PER_CORE, dw : dw + 128 : 2],
            start=False,
            stop=(stop and dw == 2),
        )


def _build_program():
    key = (MM_DTYPE, STRATEGY, NCHUNK, WARMUP)
    if key in _cache:
        return _cache[key]

    mm_dt = {"fp32": F32, "f32r": F32R, "bf16": BF16}[MM_DTYPE]
    w_dram_dt = BF16 if mm_dt is BF16 else F32

    nc = bacc.Bacc(None)
    xc = nc.declare_dram_parameter("xc", [128, B, PITCH], F32, isOutput=False)
    # One combined weight tensor: cols 0:384 = pair weights (dh=0/1 on
    # partition halves), cols 384:768 (partitions 0:64) = dh=2 weights.
    wc = nc.declare_dram_parameter("wc", [128, 2 * 3 * K], w_dram_dt, isOutput=False)
    out = nc.declare_dram_parameter("out", [K, NOUT], F32, isOutput=True)

    bpc = B // NCHUNK  # batches per DMA chunk

    with tile.TileContext(nc) as tc:
        with (
            tc.tile_pool(name="sbuf", bufs=1) as pool,
            tc.tile_pool(name="psum", bufs=1, space="PSUM") as ppool,
        ):
            wt = pool.tile([128, 2 * 3 * K], mm_dt, tag="wt")
            # f32r needs a rounding producer -> SWDGE cast DMA; bf16/fp32 are
            # plain HWDGE copies.
            wdma = nc.gpsimd if mm_dt is F32R else nc.sync
            wdma.dma_start(out=wt[:, :], in_=wc[:, :])
            wpt3 = wt[:, 0 : 3 * K].rearrange("p (a b) -> p a b", a=3)
            w2t3 = wt[:, 3 * K : 6 * K].rearrange("p (a b) -> p a b", a=3)

            staging = pool.tile([128, B * PITCH], F32, tag="staging")
            st3 = staging[:, :].rearrange("p (b x) -> p b x", b=B)
            xcr = xc[:, :, :]
            # All input chunks on ONE HWDGE queue: strict FIFO makes chunk i
            # complete before chunk i+1 starts, so the adds pipeline. (Two
            # queues interleave at packet granularity and everything lands
            # at the same late time.)
            for ch in range(NCHUNK):
                b0 = ch * bpc
                nc.sync.dma_start(
                    out=st3[:, b0 : b0 + bpc, :], in_=xcr[:, b0 : b0 + bpc, :]
                )

            psum = ppool.tile([K, NOUT], F32, tag="psum")
            psum3 = psum[:, :].rearrange("p (r w) -> p r w", r=ROWS_PER_CORE)

            if WARMUP:
                # Reads chunk-0 staging data reinterpreted as bf16 -> starts
                # as soon as the first chunk lands, keeps the PE busy (bf16
                # counts for HAM) until the real matmuls begin.
                dpsum = ppool.tile([K, NOUT], F32, tag="dpsum")
                wu_lhs = staging[:, 0:64].bitcast(BF16)
                wu_rhs = staging[:, 64:320].bitcast(BF16)
                for i in range(WARMUP):
                    nc.tensor.matmul(
                        dpsum[:, :], wu_lhs, wu_rhs,
                        start=(i == 0), stop=(i == WARMUP - 1),
                    )

            spair = pool.tile([128, 4 * PITCH], mm_dt if STRATEGY == "pair" else F32,
                              tag="spair")
            sp3 = spair[:, :].rearrange("p (b x) -> p b x", b=4)

            if STRATEGY == "pair":
                for bp in range(4):
                    nc.vector.tensor_add(
                        sp3[:, bp, :], st3[:, 2 * bp, :], st3[:, 2 * bp + 1, :]
                    )
                    src3 = sp3[:, bp, :].rearrange("p (r w) -> p r w", r=ER)
                    _emit_conv_matmuls(
                        nc, wpt3, w2t3, src3, psum3, start=(bp == 0), stop=(bp == 3)
                    )
            elif STRATEGY == "quad":
                quad = pool.tile([128, 2 * PITCH], mm_dt, tag="quad")
                q3 = quad[:, :].rearrange("p (b x) -> p b x", b=2)
                for h in range(2):
                    for bp in (2 * h, 2 * h + 1):
                        nc.vector.tensor_add(
                            sp3[:, bp, :], st3[:, 2 * bp, :], st3[:, 2 * bp + 1, :]
                        )
                    nc.vector.tensor_add(
                        q3[:, h, :], sp3[:, 2 * h, :], sp3[:, 2 * h + 1, :]
                    )
                    src3 = q3[:, h, :].rearrange("p (r w) -> p r w", r=ER)
                    _emit_conv_matmuls(
                        nc, wpt3, w2t3, src3, psum3, start=(h == 0), stop=(h == 1)
                    )
            else:  # tree
                t01 = pool.tile([128, PITCH], F32, tag="t01")
                t23 = pool.tile([128, PITCH], F32, tag="t23")
                xsum = pool.tile([128, PITCH], mm_dt, tag="xsum")
                for bp in range(2):
                    nc.vector.tensor_add(
                        sp3[:, bp, :], st3[:, 2 * bp, :], st3[:, 2 * bp + 1, :]
                    )
                nc.vector.tensor_add(t01[:, :], sp3[:, 0, :], sp3[:, 1, :])
                for bp in range(2, 4):
                    nc.vector.tensor_add(
                        sp3[:, bp, :], st3[:, 2 * bp, :], st3[:, 2 * bp + 1, :]
                    )
                nc.vector.tensor_add(t23[:, :], sp3[:, 2, :], sp3[:, 3, :])
                nc.vector.tensor_add(xsum[:, :], t01[:, :], t23[:, :])
                src3 = xsum[:, :].rearrange("p (r w) -> p r w", r=ER)
                _emit_conv_matmuls(nc, wpt3, w2t3, src3, psum3, start=True, stop=True)

            outs = pool.tile([K, NOUT], F32, tag="outs")
            nc.vector.tensor_copy(outs[:, :], psum[:, :])
            nc.sync.dma_start(out=out[:, :], in_=outs[:, :])

    nc.compile()
    _cache[key] = nc
    return nc


def _prep_inputs(input, weight):
    inp = np.ascontiguousarray(input, dtype=np.float32)
    w = np.ascontiguousarray(weight, dtype=np.float32)

    # Padded top-left region: P[r, w] = padded coord (orig r-1, w-1)
    P = np.zeros((B, C, 130, WCOLS), np.float32)
    P[:, :, 1:129, 1:129] = inp[:, :, :128, :128]
    Pc = np.ascontiguousarray(P.transpose(1, 0, 2, 3))  # (C, B, 130, WCOLS)

    wdt = _mm_np_dtype()
    t = [w[:, :, dh, :].transpose(1, 2, 0).reshape(-1, 3 * K) for dh in range(3)]
    wc_host = np.zeros((128, 6 * K), np.float32)
    wc_host[0:64, 0 : 3 * K] = t[0]
    wc_host[64:128, 0 : 3 * K] = t[1]
    wc_host[0:64, 3 * K : 6 * K] = t[2]
    wc_host = np.ascontiguousarray(wc_host.astype(wdt))

    in_maps = []
    for q in range(NCORES):
        r0 = 16 * q
        xcq = np.zeros((128, B, PITCH), np.float32)
        xcq[0:64] = Pc[:, :, r0 : r0 + 17 : 2, :].reshape(64, B, PITCH)
        xcq[64:128, :, 0 : OR * WCOLS] = Pc[:, :, r0 + 1 : r0 + 16 : 2, :].reshape(
            64, B, OR * WCOLS
        )
        in_maps.append({"xc": xcq, "wc": wc_host})
    return in_maps


def kernel(input, weight):
    global LAST_EXEC_NS
    nc = _build_program()
    in_maps = _prep_inputs(input, weight)
    res = run_bass_kernel_spmd(nc, in_maps, list(range(NCORES)), trace=TRACE)
    LAST_EXEC_NS = res.exec_time_ns

    vals = np.concatenate(
        [res.results[q]["out"].reshape(K, ROWS_PER_CORE, 64) for q in range(NCORES)],
        axis=1,
    )  # (K, 64, 64)
    out = np.zeros((B, K, 128, 128), np.float32)
    out[:, :, ::2, ::2] = vals[None]
    return out


# revision 13
# speedup vs baseline: 1.0303x; 1.0036x over previous
"""Trainium2 Bass kernel for nn_Conv2d_71476845740806.

Reference semantics (buggy naive Conv2d):
  xsum = pad(input, 1).sum(batch)                  # (1, C, 258, 258)
  conv = conv2d(xsum, weight, stride=2, VALID)     # (1, K, 128, 128)
  vals = conv[0, :, :64, :64]                      # (K, 64, 64)
  out  = zeros(B, K, 128, 128); out[:, :, ::2, ::2] = vals  (batch-replicated)

Only window starts (2i, 2j), i,j in [0,64) are used -> only padded rows/cols
0..128 of the summed image matter -> only input rows/cols 0..127.

Device strategy (8 cores, SPMD):
  - Shard the 64 output rows: core q computes rows 8q..8q+7 for ALL K=128
    filters. Needs padded rows 16q..16q+16 (17 rows) x 129 cols, all b, c.
  - Host preps one combined per-core tensor xc[128, 8, 1170]:
    partitions 0..63 = (c, even padded rows 0..8 x 130 cols),
    partitions 64..127 = (c, odd padded rows 0..7 x 130 cols, zero-padded).
    Contiguous per (partition, batch) -> near-peak DMA efficiency.
  - Batch-sum via DVE tensor_add (casting to the matmul dtype on write).
  - Conv as 6 matmuls per accumulation group into one PSUM bank [128, 512]:
      3 x contract-128 (kernel rows dh=0,1 paired across partition halves)
      3 x contract-64  (dh=2, even partitions only, shifted one row)
    rhs AP does the stride-2 column access directly: [.., (8 rows), (64 cols step 2)].
  - DMA out per-core vals (128, 8*64); host scatters into the zero output.
"""

import ml_dtypes
import numpy as np

import concourse.bacc as bacc
import concourse.bass as bass
import concourse.mybir as mybir
from concourse import tile
from concourse.bass_utils import run_bass_kernel_spmd

F32 = mybir.dt.float32
F32R = mybir.dt.float32r
BF16 = mybir.dt.bfloat16

B, C, H, W = 8, 64, 256, 256
K = 128
NCORES = 8
ROWS_PER_CORE = 8          # output rows per core (64 total)
ER = 9                     # even padded rows per core
OR = 8                     # odd padded rows per core
WCOLS = 130                # stored padded cols 0..129 (used: 0..128)
PITCH = ER * WCOLS         # 1170 per-batch free pitch
NOUT = ROWS_PER_CORE * 64  # 512

# Matmul input dtype: "fp32" (exact, 4 cyc/row), "f32r" (1 cyc/row),
# "bf16" (1 cyc/row, HAM-warmable).
MM_DTYPE = "f32r"
# Sum strategy: "tree" = full sum then 6 matmuls; "quad" = 2 groups of 4
# batches (12 matmuls); "pair" = 4 groups of 2 batches (24 matmuls).
STRATEGY = "pair"
# Input DMA chunks (must divide 8): 4 = batch pairs, 8 = single batches.
NCHUNK = 8
# Dummy bf16 matmuls (on already-landed chunk-0 data) to lift the PE HAM
# clock gate to 8/8 before the real matmuls start. 0 = off.
WARMUP = 0

TRACE = False
LAST_EXEC_NS = None

_cache = {}


def _mm_np_dtype():
    return ml_dtypes.bfloat16 if MM_DTYPE == "bf16" else np.float32


def _emit_conv_matmuls(nc, wpt3, w2t3, src3, psum3, start, stop):
    """6 matmuls of the 3x3 stride-2 conv of src into psum (accumulating).

    src3: [128, ER, WCOLS] AP; partitions 0..63 = (c, even rows),
          64..127 = (c, odd rows; row index i is padded row 2i+1).
    """
    for dw in range(3):
        # dh=0 (even rows, row i) paired with dh=1 (odd rows, row i)
        nc.tensor.matmul(
            psum3[:, :, :],
            wpt3[:, dw, :],
            src3[:, 0:ROWS_PER_CORE, dw : dw + 128 : 2],
            start=(start and dw == 0),
            stop=False,
        )
    for dw in range(3):
        # dh=2: even rows, row i+1
        nc.tensor.matmul(
            psum3[0:K, :, :],
            w2t3[0:64, dw, :],
            src3[0:64, 1 : 1 + ROWS_PER_CORE, dw : dw + 128 : 2],
            start=False,
            stop=(stop and dw == 2),
        )


def _build_program_raw():
    """Hand-synchronized bacc version of the f32r/pair pipeline — no
    TileContext, so no end-of-kernel semaphore-reset storm."""
    import contextlib

    nc = bacc.Bacc(None)
    xc = nc.declare_dram_parameter("xc", [128, B, PITCH], F32, isOutput=False)
    wc = nc.declare_dram_parameter("wc", [128, 2 * 3 * K], F32, isOutput=False)
    out = nc.declare_dram_parameter("out", [K, NOUT], F32, isOutput=True)

    ctx = contextlib.ExitStack()
    wt = ctx.enter_context(nc.sbuf_tensor([128, 2 * 3 * K], F32R))
    staging = ctx.enter_context(nc.sbuf_tensor([128, B * PITCH], F32))
    spair = ctx.enter_context(nc.sbuf_tensor([128, 4 * PITCH], F32R))
    outs = ctx.enter_context(nc.sbuf_tensor([K, NOUT], F32))
    psum = ctx.enter_context(nc.psum_tensor([K, NOUT], F32))
    in_sem = ctx.enter_context(nc.semaphore("in_sem"))
    w_sem = ctx.enter_context(nc.semaphore("w_sem"))
    add_sem = ctx.enter_context(nc.semaphore("add_sem"))
    mm_sem = ctx.enter_context(nc.semaphore("mm_sem"))
    cp_sem = ctx.enter_context(nc.semaphore("cp_sem"))
    odma_sem = ctx.enter_context(nc.semaphore("odma_sem"))

    wpt3 = wt[:, 0 : 3 * K].rearrange("p (a b) -> p a b", a=3)
    w2t3 = wt[:, 3 * K : 6 * K].rearrange("p (a b) -> p a b", a=3)
    st3 = staging[:, :].rearrange("p (b x) -> p b x", b=B)
    sp3 = spair[:, :].rearrange("p (b x) -> p b x", b=4)
    psum3 = psum[:, :].rearrange("p (r w) -> p r w", r=ROWS_PER_CORE)

    with nc.Block() as block:

        @block.gpsimd
        def _(g):
            # SWDGE cast DMA rounds fp32 -> f32r for the verifier.
            g.dma_start(out=wt[:, :], in_=wc[:, :]).then_inc(w_sem, 16)

        @block.sync
        def _(sync):
            for ch in range(B):
                sync.dma_start(
                    out=st3[:, ch : ch + 1, :], in_=xc[:, ch : ch + 1, :]
                ).then_inc(in_sem, 16)
            sync.wait_ge(cp_sem, 1)
            sync.dma_start(out=out[:, :], in_=outs[:, :]).then_inc(odma_sem, 16)
            sync.wait_ge(odma_sem, 16)

        @block.vector
        def _(v):
            for bp in range(4):
                v.wait_ge(in_sem, (2 * bp + 2) * 16)
                v.tensor_add(
                    sp3[:, bp, :], st3[:, 2 * bp, :], st3[:, 2 * bp + 1, :]
                ).then_inc(add_sem, 1)
            v.wait_ge(mm_sem, 1)
            v.tensor_copy(outs[:, :], psum[:, :]).then_inc(cp_sem, 1)

        @block.tensor
        def _(t):
            t.wait_ge(w_sem, 16)
            for bp in range(4):
                t.wait_ge(add_sem, bp + 1)
                src3 = sp3[:, bp, :].rearrange("p (r w) -> p r w", r=ER)
                for dw in range(3):
                    nc.tensor.matmul(
                        psum3[:, :, :],
                        wpt3[:, dw, :],
                        src3[:, 0:ROWS_PER_CORE, dw : dw + 128 : 2],
                        start=(bp == 0 and dw == 0),
                        stop=False,
                    )
                for dw in range(3):
                    mm = nc.tensor.matmul(
                        psum3[0:K, :, :],
                        w2t3[0:64, dw, :],
                        src3[0:64, 1 : 1 + ROWS_PER_CORE, dw : dw + 128 : 2],
                        start=False,
                        stop=(bp == 3 and dw == 2),
                    )
                    if bp == 3 and dw == 2:
                        mm.then_inc(mm_sem, 1)

    nc.compile()
    ctx.close()
    return nc


def _build_program():
    key = (MM_DTYPE, STRATEGY, NCHUNK, WARMUP)
    if key in _cache:
        return _cache[key]
    if STRATEGY == "raw":
        nc = _build_program_raw()
        _cache[key] = nc
        return nc

    mm_dt = {"fp32": F32, "f32r": F32R, "bf16": BF16}[MM_DTYPE]
    w_dram_dt = BF16 if mm_dt is BF16 else F32

    nc = bacc.Bacc(None)
    xc = nc.declare_dram_parameter("xc", [128, B, PITCH], F32, isOutput=False)
    # One combined weight tensor: cols 0:384 = pair weights (dh=0/1 on
    # partition halves), cols 384:768 (partitions 0:64) = dh=2 weights.
    wc = nc.declare_dram_parameter("wc", [128, 2 * 3 * K], w_dram_dt, isOutput=False)
    out = nc.declare_dram_parameter("out", [K, NOUT], F32, isOutput=True)

    bpc = B // NCHUNK  # batches per DMA chunk

    with tile.TileContext(nc) as tc:
        with (
            tc.tile_pool(name="sbuf", bufs=1) as pool,
            tc.tile_pool(name="psum", bufs=1, space="PSUM") as ppool,
        ):
            wt = pool.tile([128, 2 * 3 * K], mm_dt, tag="wt")
            # f32r needs a rounding producer -> SWDGE cast DMA; bf16/fp32 are
            # plain HWDGE copies.
            wdma = nc.gpsimd if mm_dt is F32R else nc.sync
            wdma.dma_start(out=wt[:, :], in_=wc[:, :])
            wpt3 = wt[:, 0 : 3 * K].rearrange("p (a b) -> p a b", a=3)
            w2t3 = wt[:, 3 * K : 6 * K].rearrange("p (a b) -> p a b", a=3)

            staging = pool.tile([128, B * PITCH], F32, tag="staging")
            st3 = staging[:, :].rearrange("p (b x) -> p b x", b=B)
            xcr = xc[:, :, :]
            # All input chunks on ONE HWDGE queue: strict FIFO makes chunk i
            # complete before chunk i+1 starts, so the adds pipeline. (Two
            # queues interleave at packet granularity and everything lands
            # at the same late time.)
            for ch in range(NCHUNK):
                b0 = ch * bpc
                nc.sync.dma_start(
                    out=st3[:, b0 : b0 + bpc, :], in_=xcr[:, b0 : b0 + bpc, :]
                )

            psum = ppool.tile([K, NOUT], F32, tag="psum")
            psum3 = psum[:, :].rearrange("p (r w) -> p r w", r=ROWS_PER_CORE)

            if WARMUP:
                # Reads chunk-0 staging data reinterpreted as bf16 -> starts
                # as soon as the first chunk lands, keeps the PE busy (bf16
                # counts for HAM) until the real matmuls begin.
                dpsum = ppool.tile([K, NOUT], F32, tag="dpsum")
                wu_lhs = staging[:, 0:64].bitcast(BF16)
                wu_rhs = staging[:, 64:320].bitcast(BF16)
                for i in range(WARMUP):
                    nc.tensor.matmul(
                        dpsum[:, :], wu_lhs, wu_rhs,
                        start=(i == 0), stop=(i == WARMUP - 1),
                    )

            spair = pool.tile([128, 4 * PITCH], mm_dt if STRATEGY == "pair" else F32,
                              tag="spair")
            sp3 = spair[:, :].rearrange("p (b x) -> p b x", b=4)

            if STRATEGY == "pair":
                for bp in range(4):
                    nc.vector.tensor_add(
                        sp3[:, bp, :], st3[:, 2 * bp, :], st3[:, 2 * bp + 1, :]
                    )
                    src3 = sp3[:, bp, :].rearrange("p (r w) -> p r w", r=ER)
                    _emit_conv_matmuls(
                        nc, wpt3, w2t3, src3, psum3, start=(bp == 0), stop=(bp == 3)
                    )
            elif STRATEGY == "quad":
                quad = pool.tile([128, 2 * PITCH], mm_dt, tag="quad")
                q3 = quad[:, :].rearrange("p (b x) -> p b x", b=2)
                for h in range(2):
                    for bp in (2 * h, 2 * h + 1):
                        nc.vector.tensor_add(
                            sp3[:, bp, :], st3[:, 2 * bp, :], st3[:, 2 * bp + 1, :]
                        )
                    nc.vector.tensor_add(
                        q3[:, h, :], sp3[:, 2 * h, :], sp3[:, 2 * h + 1, :]
                    )
                    src3 = q3[:, h, :].rearrange("p (r w) -> p r w", r=ER)
                    _emit_conv_matmuls(
                        nc, wpt3, w2t3, src3, psum3, start=(h == 0), stop=(h == 1)
                    )
            else:  # tree
                t01 = pool.tile([128, PITCH], F32, tag="t01")
                t23 = pool.tile([128, PITCH], F32, tag="t23")
                xsum = pool.tile([128, PITCH], mm_dt, tag="xsum")
                for bp in range(2):
                    nc.vector.tensor_add(
                        sp3[:, bp, :], st3[:, 2 * bp, :], st3[:, 2 * bp + 1, :]
                    )
                nc.vector.tensor_add(t01[:, :], sp3[:, 0, :], sp3[:, 1, :])
                for bp in range(2, 4):
                    nc.vector.tensor_add(
                        sp3[:, bp, :], st3[:, 2 * bp, :], st3[:, 2 * bp + 1, :]
                    )
                nc.vector.tensor_add(t23[:, :], sp3[:, 2, :], sp3[:, 3, :])
                nc.vector.tensor_add(xsum[:, :], t01[:, :], t23[:, :])
                src3 = xsum[:, :].rearrange("p (r w) -> p r w", r=ER)
                _emit_conv_matmuls(nc, wpt3, w2t3, src3, psum3, start=True, stop=True)

            outs = pool.tile([K, NOUT], F32, tag="outs")
            nc.vector.tensor_copy(outs[:, :], psum[:, :])
            nc.sync.dma_start(out=out[:, :], in_=outs[:, :])

    nc.compile()
    _cache[key] = nc
    return nc


def _prep_inputs(input, weight):
    inp = np.ascontiguousarray(input, dtype=np.float32)
    w = np.ascontiguousarray(weight, dtype=np.float32)

    # Padded top-left region: P[r, w] = padded coord (orig r-1, w-1)
    P = np.zeros((B, C, 130, WCOLS), np.float32)
    P[:, :, 1:129, 1:129] = inp[:, :, :128, :128]
    Pc = np.ascontiguousarray(P.transpose(1, 0, 2, 3))  # (C, B, 130, WCOLS)

    wdt = _mm_np_dtype()
    t = [w[:, :, dh, :].transpose(1, 2, 0).reshape(-1, 3 * K) for dh in range(3)]
    wc_host = np.zeros((128, 6 * K), np.float32)
    wc_host[0:64, 0 : 3 * K] = t[0]
    wc_host[64:128, 0 : 3 * K] = t[1]
    wc_host[0:64, 3 * K : 6 * K] = t[2]
    wc_host = np.ascontiguousarray(wc_host.astype(wdt))

    in_maps = []
    for q in range(NCORES):
        r0 = 16 * q
        xcq = np.zeros((128, B, PITCH), np.float32)
        xcq[0:64] = Pc[:, :, r0 : r0 + 17 : 2, :].reshape(64, B, PITCH)
        xcq[64:128, :, 0 : OR * WCOLS] = Pc[:, :, r0 + 1 : r0 + 16 : 2, :].reshape(
            64, B, OR * WCOLS
        )
        in_maps.append({"xc": xcq, "wc": wc_host})
    return in_maps


def kernel(input, weight):
    global LAST_EXEC_NS
    nc = _build_program()
    in_maps = _prep_inputs(input, weight)
    res = run_bass_kernel_spmd(nc, in_maps, list(range(NCORES)), trace=TRACE)
    LAST_EXEC_NS = res.exec_time_ns

    vals = np.concatenate(
        [res.results[q]["out"].reshape(K, ROWS_PER_CORE, 64) for q in range(NCORES)],
        axis=1,
    )  # (K, 64, 64)
    out = np.zeros((B, K, 128, 128), np.float32)
    out[:, :, ::2, ::2] = vals[None]
    return out
